# revision 19
# baseline (speedup 1.0000x reference)
# Trainium2 Bass kernel for nn_MultiCondLayer:
#   out[b,o,n] = (sum_k (cond[b] @ W[k].T)[o,n] + sum_k b[k,o]) * x_mask[b,0,n]
# Algebraic reduction: sum_k Linear_k(x) == Linear(x) with W' = sum_k W[k],
# b' = sum_k b[k]  (4x FLOP reduction vs. the naive einsum over k).
#
# Sharding: data-parallel over batch B=8 across the 8 NeuronCores (one batch
# element per core); the reduced [1024,1024] weight is replicated.
#
# Precision: all operands are cast to bf16 on the host (x, W', mask) and the
# output is stored bf16 and upcast on the host. PSUM accumulation stays fp32.
# The PE streams bf16 at the same 1 col/cycle as fp32r, so this does not
# change the ~110us matmul floor, but it halves HBM traffic 38->19 MB/core,
# enables FWL (fp32 LDWEIGHTS was ~330ns and leaked into the issue rate;
# bf16 is ~97ns, fully hidden), and halves the startup ramp and store tail.
# End-to-end rel err ~3e-3, well under the 2e-2 gate.
#
# Schedule notes (from perfetto/NTFF iterations):
# - The PE HAM clock gate needs ~3.4us of UNBROKEN matmul activity to go
#   1.2->2.4GHz, and any idle gap re-throttles it. So the kernel opens with
#   dummy matmuls on memset data (no DMA dependency), rolls into the mask
#   PE-broadcast, then into the main chains, with no PE gap anywhere.
# - All in-flight DMAs share the ~358 GB/s HBM limit round-robin regardless
#   of ring/queue, so the startup-critical set (mask row 8KB, x window 0
#   1MB, first w o-pair column 512KB) is issued alone; every other load is
#   gated behind a tiny 1-element memset placed on the vector queue (WAW
#   overlap -> the DMA's issue waits for it), timed so each window lands
#   just before its chains.
# - Outputs are evicted per chain by fused DVE (psum+bias)*mask into wide
#   per-window [128, 8o, nw] tiles and stored as ONE DMA per window
#   (automatically gated by the window's last eviction), so no store
#   traffic competes with the startup loads. The last window stores per
#   chain with a split final store to minimize the end-of-kernel tail.
# - Main stream: per n-window, 8 serial o-chains of 8 c-steps (LDW + 1-2
#   matmuls per step) accumulating into 1-2 psum banks; evictions stagger
#   and psum bank reuse has ~4 chains of slack. Measured issue rate:
#   216ns/matmul (the N=512 floor).

import numpy as np

import ml_dtypes

import concourse.bass as bass
import concourse.mybir as mybir
import concourse.tile as tile
from concourse import bacc
from concourse.bass_utils import run_bass_kernel_spmd

P = 128
B, C, N = 8, 1024, 4096
O = 1024
NT = 512                 # matmul free dim = one fp32 PSUM bank
CO, OO = C // P, O // P
# n-window plan: narrow first window so the startup-critical x DMA is only
# 1MB; narrow last window so the final evict+store tail is small.
WINDOWS = [512, 1024, 1024, 1024, 512]
F32 = mybir.dt.float32
BF16 = mybir.dt.bfloat16

N_CORES = 8
N_WARM = 9               # dummy matmuls to warm the HAM clock gate
OP = 2 * P               # w load granularity: o-pair columns (512KB)


def build_module():
    nc = bacc.Bacc("TRN2", target_bir_lowering=False, debug=False,
                   num_devices=N_CORES)
    x = nc.dram_tensor("x", [C, N], BF16, kind="ExternalInput")    # cond[b]
    wt = nc.dram_tensor("wt", [C, O], BF16, kind="ExternalInput")  # (sum_k W[k]).T
    # bias pre-transposed on host to [128, OO]: 128 contiguous rows.
    bv = nc.dram_tensor("bv", [P, OO], F32, kind="ExternalInput")
    mk = nc.dram_tensor("mk", [N], BF16, kind="ExternalInput")     # x_mask[b,0]
    out = nc.dram_tensor("out", [O, N], BF16, kind="ExternalOutput")

    x_r = x.ap().rearrange("(c p) n -> p c n", p=P)      # [128, CO, N]
    wt_r = wt.ap().rearrange("(c p) o -> p c o", p=P)    # [128, CO, O]
    out_r = out.ap().rearrange("(o p) n -> p o n", p=P)  # [128, OO, N]

    win_off = []
    n0 = 0
    for nw in WINDOWS:
        win_off.append(n0)
        n0 += nw

    with tile.TileContext(nc) as tc:
        with (
            tc.tile_pool(name="consts", bufs=1) as consts,
            tc.tile_pool(name="outs", bufs=4) as outs,
            tc.tile_pool(name="ps", bufs=8, space="PSUM") as psp,
        ):
            # --- The 8KB mask row must complete EARLY (it gates the mask
            # broadcast that keeps the PE ramp unbroken), and a
            # single-partition DMA's completion sem straggles behind any
            # concurrent bulk packets -- so it goes absolutely first, on
            # the otherwise-idle gpsimd SWDGE queue, before anything else
            # enters the rings. Bias rides behind it.
            mkrow_sb = consts.tile([1, N], BF16)
            nc.gpsimd.dma_start(mkrow_sb[:], mk.ap()[None, :])
            bias_sb = consts.tile([P, OO], F32)
            nc.gpsimd.dma_start(bias_sb[:], bv.ap())
            ones_sb = consts.tile([1, P], BF16)
            nc.gpsimd.memset(ones_sb[:], 1.0)
            # warmup scratch on vector (gates the first dummy matmul)
            scratch = consts.tile([P, NT], BF16)
            nc.vector.memset(scratch[:], 0.0)

            # --- startup-critical loads: x window 0 (1MB) on sync, first
            # w o-pair column (512KB) on scalar.
            x_sb = consts.tile([P, CO, N], BF16)
            nc.sync.dma_start(x_sb[:, :, 0:WINDOWS[0]],
                              x_r[:, :, 0:WINDOWS[0]])
            w_sb = consts.tile([P, CO, O], BF16)
            nc.scalar.dma_start(w_sb[:, :, 0:OP], wt_r[:, :, 0:OP])

            # Deferred-load helper: a 1-element vector copy that READS a
            # tile produced by `after` (a real data dependency -- the Tile
            # scheduler hoists dependency-free ops) and writes the DMA's
            # destination byte, so the DMA's issue waits (WAW) until
            # `after` has executed -- keeping bulk loads out of the
            # startup-critical HBM window.
            def gated_load(engine, dst, src, after_1elem):
                g = dst[0:1, 0, 0:1]
                nc.vector.tensor_copy(g, after_1elem)
                engine.dma_start(dst, src)

            # --- PE warmup: no-DMA dummy matmuls release the HAM throttle
            # (cold 1.2GHz -> warm 2.4GHz needs ~3.4us of UNBROKEN busy)
            # while the critical loads are in flight. ---
            for i in range(N_WARM):
                wps = psp.tile([P, NT], F32, name=f"warm_{i}", tag="ps")
                nc.tensor.matmul(wps[:], scratch[:, 0:P], scratch[:],
                                 start=True, stop=True)

            # --- Mask broadcast on-chip: ones[128,1] (x) mkrow[1,N] via PE
            # (keeps warming; avoids a 128x replicated mask DMA). Bulk w
            # columns and x window 1 unlock as the copies retire. ---
            mask_sb = consts.tile([P, N], BF16)
            for n in range(N // NT):
                mps = psp.tile([P, NT], F32, name=f"mps_{n}", tag="ps")
                nc.tensor.matmul(mps[:], ones_sb[:],
                                 mkrow_sb[:, n * NT:(n + 1) * NT],
                                 start=True, stop=True)
                nc.vector.tensor_copy(mask_sb[:, n * NT:(n + 1) * NT], mps[:])
                if n == 0:
                    # second w o-pair column unlocks on the first mask copy
                    gated_load(nc.scalar, w_sb[:, :, OP:2 * OP],
                               wt_r[:, :, OP:2 * OP],
                               mask_sb[0:1, 0:1])

            # --- Main stream: per n-window, 8 serial o-chains of 8 c-steps.
            last_ns = len(WINDOWS) - 1
            for ns, nw in enumerate(WINDOWS):
                n0 = win_off[ns]
                nsub = nw // NT
                if ns != last_ns:
                    ot_w = outs.tile([P, OO, nw], BF16, name=f"otw_{ns}",
                                     tag=f"otw{nw}", bufs=2)
                for o in range(OO):
                    pss = [psp.tile([P, NT], F32, name=f"ps_{ns}_{o}_{j}",
                                    tag="ps") for j in range(nsub)]
                    for c in range(CO):
                        w_ap = w_sb[:, c, o * P:(o + 1) * P]
                        for j in range(nsub):
                            nj = n0 + j * NT
                            nc.tensor.matmul(pss[j][:], w_ap,
                                             x_sb[:, c, nj:nj + NT],
                                             start=(c == 0),
                                             stop=(c == CO - 1))
                    if ns == last_ns:
                        # last window: per-chain stores; split the final
                        # store so its completion receipt starts earlier.
                        ot = outs.tile([P, nw], BF16, name=f"ot_{ns}_{o}",
                                       tag="ot_last", bufs=4)
                        if o == OO - 1:
                            H = nw // 2
                            nc.vector.scalar_tensor_tensor(
                                ot[:, 0:H], pss[0][:, 0:H],
                                bias_sb[:, o:o + 1], mask_sb[:, n0:n0 + H],
                                op0=mybir.AluOpType.add,
                                op1=mybir.AluOpType.mult)
                            nc.scalar.dma_start(
                                out.ap()[o * P:(o + 1) * P, n0:n0 + H],
                                ot[:, 0:H])
                            nc.vector.scalar_tensor_tensor(
                                ot[:, H:nw], pss[0][:, H:nw],
                                bias_sb[:, o:o + 1],
                                mask_sb[:, n0 + H:n0 + nw],
                                op0=mybir.AluOpType.add,
                                op1=mybir.AluOpType.mult)
                            nc.scalar.dma_start(
                                out.ap()[o * P:(o + 1) * P, n0 + H:n0 + nw],
                                ot[:, H:nw])
                        else:
                            nc.vector.scalar_tensor_tensor(
                                ot[:, 0:nw], pss[0][:],
                                bias_sb[:, o:o + 1], mask_sb[:, n0:n0 + nw],
                                op0=mybir.AluOpType.add,
                                op1=mybir.AluOpType.mult)
                            nc.scalar.dma_start(
                                out.ap()[o * P:(o + 1) * P, n0:n0 + nw],
                                ot[:])
                    else:
                        for j in range(nsub):
                            nj = n0 + j * NT
                            nc.vector.scalar_tensor_tensor(
                                ot_w[:, o, j * NT:(j + 1) * NT], pss[j][:],
                                bias_sb[:, o:o + 1], mask_sb[:, nj:nj + NT],
                                op0=mybir.AluOpType.add,
                                op1=mybir.AluOpType.mult)
                    # deferred bulk loads, unlocked by eviction progress
                    # (gate reads one element the chain's eviction wrote),
                    # spaced so each lands just before its deadline and
                    # never floods HBM alongside an earlier transfer.
                    if ns == 0 and o in (0, 2):
                        op = 2 + o // 2
                        gated_load(nc.scalar,
                                   w_sb[:, :, op * OP:(op + 1) * OP],
                                   wt_r[:, :, op * OP:(op + 1) * OP],
                                   ot_w[0:1, o, 0:1])
                    gate_w = None
                    if ns == 0 and o == 3:
                        gate_w = 1
                    elif ns == 0 and o == 7:
                        gate_w = 2
                    elif ns == 1 and o == 2:
                        gate_w = 3
                    elif ns == 1 and o == 6:
                        gate_w = 4
                    if gate_w is not None:
                        gw0 = win_off[gate_w]
                        gnw = WINDOWS[gate_w]
                        gated_load(nc.sync,
                                   x_sb[:, :, gw0:gw0 + gnw],
                                   x_r[:, :, gw0:gw0 + gnw],
                                   ot_w[0:1, o, 0:1])
                if ns != last_ns:
                    # one wide store for the whole window, gated by its
                    # last eviction (reads the full ot_w tile).
                    nc.scalar.dma_start(out_r[:, :, n0:n0 + nw], ot_w[:])
    nc.compile()
    return nc


_NC_CACHE = None


def _get_module():
    global _NC_CACHE
    if _NC_CACHE is None:
        _NC_CACHE = build_module()
    return _NC_CACHE


def _make_in_maps(cond, x_mask, W, b):
    bf16 = ml_dtypes.bfloat16
    wt = np.ascontiguousarray(
        W.astype(np.float32).sum(axis=0).T.astype(bf16))           # [C, O]
    bv = np.ascontiguousarray(
        b.astype(np.float32).sum(axis=0).reshape(OO, P).T,
        dtype=np.float32)                                          # [128, OO]
    in_maps = []
    for core in range(N_CORES):
        in_maps.append({
            "x": np.ascontiguousarray(cond[core].astype(bf16)),
            "wt": wt,
            "bv": bv,
            "mk": np.ascontiguousarray(x_mask[core, 0].astype(bf16)),
        })
    return in_maps


def run(cond, x_mask, W, b, trace=False, trace_cores=None):
    """Run on hardware; returns (out [B,O,N] fp32, BassKernelResults)."""
    nc = _get_module()
    in_maps = _make_in_maps(cond, x_mask, W, b)
    res = run_bass_kernel_spmd(
        nc, in_maps, core_ids=list(range(N_CORES)),
        trace=trace, trace_cores=trace_cores,
    )
    out = np.stack(
        [res.results[i]["out"].astype(np.float32) for i in range(N_CORES)],
        axis=0)
    return out, res


def kernel(cond, x_mask, W, b):
    out, _ = run(cond, x_mask, W, b)
    return out


# revision 23
# speedup vs baseline: 1.1767x; 1.1767x over previous
# Trainium2 Bass kernel for nn_MultiCondLayer:
#   out[b,o,n] = (sum_k (cond[b] @ W[k].T)[o,n] + sum_k b[k,o]) * x_mask[b,0,n]
# Algebraic reduction: sum_k Linear_k(x) == Linear(x) with W' = sum_k W[k],
# b' = sum_k b[k]  (4x FLOP reduction vs. the naive einsum over k).
#
# Sharding: data-parallel over batch B=8 across the 8 NeuronCores (one batch
# element per core); the reduced [1024,1024] weight is replicated.
#
# Precision: all operands are cast to bf16 on the host (x, W', mask) and the
# output is stored bf16 and upcast on the host. PSUM accumulation stays fp32.
# The PE streams bf16 at the same 1 col/cycle as fp32r, so this does not
# change the ~110us matmul floor, but it halves HBM traffic 38->19 MB/core,
# enables FWL (fp32 LDWEIGHTS was ~330ns and leaked into the issue rate;
# bf16 is ~97ns, fully hidden), and halves the startup ramp and store tail.
# End-to-end rel err ~3e-3, well under the 2e-2 gate.
#
# Schedule notes (from perfetto/NTFF iterations):
# - The PE HAM clock gate needs ~3.4us of UNBROKEN matmul activity to go
#   1.2->2.4GHz, and any idle gap re-throttles it. So the kernel opens with
#   dummy matmuls on memset data (no DMA dependency), rolls into the mask
#   PE-broadcast, then into the main chains, with no PE gap anywhere.
# - All in-flight DMAs share the ~358 GB/s HBM limit round-robin regardless
#   of ring/queue, so the startup-critical set (mask row 8KB, x window 0
#   1MB, first w o-pair column 512KB) is issued alone; every other load is
#   gated behind a tiny 1-element memset placed on the vector queue (WAW
#   overlap -> the DMA's issue waits for it), timed so each window lands
#   just before its chains.
# - Outputs are evicted per chain by fused DVE (psum+bias)*mask into wide
#   per-window [128, 8o, nw] tiles and stored as ONE DMA per window
#   (automatically gated by the window's last eviction), so no store
#   traffic competes with the startup loads. The last window stores per
#   chain with a split final store to minimize the end-of-kernel tail.
# - Main stream: per n-window, 8 serial o-chains of 8 c-steps (LDW + 1-2
#   matmuls per step) accumulating into 1-2 psum banks; evictions stagger
#   and psum bank reuse has ~4 chains of slack. Measured issue rate:
#   216ns/matmul (the N=512 floor).

import numpy as np

import ml_dtypes

import concourse.bass as bass
import concourse.mybir as mybir
import concourse.tile as tile
from concourse import bacc
from concourse.bass_utils import run_bass_kernel_spmd

P = 128
B, C, N = 8, 1024, 4096
O = 1024
NT = 512                 # matmul free dim = one fp32 PSUM bank
CO, OO = C // P, O // P
# n-window plan: narrow first window so the startup-critical x DMA is only
# 1MB; narrow last window so the final evict+store tail is small.
WINDOWS = [512, 1024, 1024, 1024, 512]
F32 = mybir.dt.float32
BF16 = mybir.dt.bfloat16

N_CORES = 8
N_WARM = 9               # dummy matmuls to warm the HAM clock gate
OP = 2 * P               # w load granularity: o-pair columns (512KB)


def build_module():
    nc = bacc.Bacc("TRN2", target_bir_lowering=False, debug=False,
                   num_devices=N_CORES)
    x = nc.dram_tensor("x", [C, N], BF16, kind="ExternalInput")    # cond[b]
    wt = nc.dram_tensor("wt", [C, O], BF16, kind="ExternalInput")  # (sum_k W[k]).T
    # bias pre-transposed on host to [128, OO]: 128 contiguous rows.
    bv = nc.dram_tensor("bv", [P, OO], F32, kind="ExternalInput")
    # mask row, divided by 16 and replicated to 16 partitions on the host:
    # a 16-partition 128KB DMA completes fast and balanced (a 1-partition
    # row DMA's completion sem straggles behind bulk packets), and the PE
    # broadcast ones[16,:].T @ mkr sums the 16 copies back to the mask.
    mkr = nc.dram_tensor("mkr", [16, N], BF16, kind="ExternalInput")
    out = nc.dram_tensor("out", [O, N], BF16, kind="ExternalOutput")

    x_r = x.ap().rearrange("(c p) n -> p c n", p=P)      # [128, CO, N]
    wt_r = wt.ap().rearrange("(c p) o -> p c o", p=P)    # [128, CO, O]
    out_r = out.ap().rearrange("(o p) n -> p o n", p=P)  # [128, OO, N]

    win_off = []
    n0 = 0
    for nw in WINDOWS:
        win_off.append(n0)
        n0 += nw

    with tile.TileContext(nc) as tc:
        with (
            tc.tile_pool(name="consts", bufs=1) as consts,
            tc.tile_pool(name="outs", bufs=4) as outs,
            tc.tile_pool(name="ps", bufs=8, space="PSUM") as psp,
        ):
            # --- gpsimd: warmup memsets (they gate the PE ramp; gpsimd is
            # free earliest and issues no DMAs).
            scratch = consts.tile([P, NT], BF16)
            nc.gpsimd.memset(scratch[:], 0.0)
            ones_sb = consts.tile([16, P], BF16)
            nc.gpsimd.memset(ones_sb[:], 1.0)

            # --- startup-critical loads: mask row + bias first on scalar
            # (tiny), then the first w o-pair column; x window 0 (1MB) on
            # sync.
            mkr_sb = consts.tile([16, N], BF16)
            nc.scalar.dma_start(mkr_sb[:], mkr.ap())
            bias_sb = consts.tile([P, OO], F32)
            nc.scalar.dma_start(bias_sb[:], bv.ap())
            x_sb = consts.tile([P, CO, N], BF16)
            nc.sync.dma_start(x_sb[:, :, 0:WINDOWS[0]],
                              x_r[:, :, 0:WINDOWS[0]])
            w_sb = consts.tile([P, CO, O], BF16)
            nc.scalar.dma_start(w_sb[:, :, 0:OP], wt_r[:, :, 0:OP])

            # Deferred-load helper: a 1-element vector copy that READS a
            # tile produced by `after` (a real data dependency -- the Tile
            # scheduler hoists dependency-free ops) and writes the DMA's
            # destination byte, so the DMA's issue waits (WAW) until
            # `after` has executed -- keeping bulk loads out of the
            # startup-critical HBM window.
            def gated_load(engine, dst, src, after_1elem):
                g = dst[0:1, 0, 0:1]
                nc.vector.tensor_copy(g, after_1elem)
                engine.dma_start(dst, src)

            # --- PE warmup: no-DMA dummy matmuls release the HAM throttle
            # (cold 1.2GHz -> warm 2.4GHz needs ~3.4us of UNBROKEN busy)
            # while the critical loads are in flight. ---
            for i in range(N_WARM):
                wps = psp.tile([P, NT], F32, name=f"warm_{i}", tag="ps")
                nc.tensor.matmul(wps[:], scratch[:, 0:P], scratch[:],
                                 start=True, stop=True)

            # --- Mask broadcast on-chip: ones[128,1] (x) mkrow[1,N] via PE
            # (keeps warming; avoids a 128x replicated mask DMA). Bulk w
            # columns and x window 1 unlock as the copies retire. ---
            mask_sb = consts.tile([P, N], BF16)
            for n in range(N // NT):
                mps = psp.tile([P, NT], F32, name=f"mps_{n}", tag="ps")
                nc.tensor.matmul(mps[:], ones_sb[:],
                                 mkr_sb[:, n * NT:(n + 1) * NT],
                                 start=True, stop=True)
                nc.vector.tensor_copy(mask_sb[:, n * NT:(n + 1) * NT], mps[:])
                if n == 0:
                    # second w o-pair column unlocks on the first mask copy
                    gated_load(nc.scalar, w_sb[:, :, OP:2 * OP],
                               wt_r[:, :, OP:2 * OP],
                               mask_sb[0:1, 0:1])

            # --- Main stream: per n-window, 8 serial o-chains of 8 c-steps.
            last_ns = len(WINDOWS) - 1
            for ns, nw in enumerate(WINDOWS):
                n0 = win_off[ns]
                nsub = nw // NT
                if ns != last_ns:
                    ot_w = outs.tile([P, OO, nw], BF16, name=f"otw_{ns}",
                                     tag=f"otw{nw}", bufs=2)
                for o in range(OO):
                    pss = [psp.tile([P, NT], F32, name=f"ps_{ns}_{o}_{j}",
                                    tag="ps") for j in range(nsub)]
                    for c in range(CO):
                        w_ap = w_sb[:, c, o * P:(o + 1) * P]
                        for j in range(nsub):
                            nj = n0 + j * NT
                            nc.tensor.matmul(pss[j][:], w_ap,
                                             x_sb[:, c, nj:nj + NT],
                                             start=(c == 0),
                                             stop=(c == CO - 1))
                    if ns == last_ns:
                        # last window: per-chain stores; split the final
                        # store so its completion receipt starts earlier.
                        ot = outs.tile([P, nw], BF16, name=f"ot_{ns}_{o}",
                                       tag="ot_last", bufs=4)
                        if o == OO - 1:
                            H = nw // 2
                            nc.vector.scalar_tensor_tensor(
                                ot[:, 0:H], pss[0][:, 0:H],
                                bias_sb[:, o:o + 1], mask_sb[:, n0:n0 + H],
                                op0=mybir.AluOpType.add,
                                op1=mybir.AluOpType.mult)
                            nc.scalar.dma_start(
                                out.ap()[o * P:(o + 1) * P, n0:n0 + H],
                                ot[:, 0:H])
                            nc.vector.scalar_tensor_tensor(
                                ot[:, H:nw], pss[0][:, H:nw],
                                bias_sb[:, o:o + 1],
                                mask_sb[:, n0 + H:n0 + nw],
                                op0=mybir.AluOpType.add,
                                op1=mybir.AluOpType.mult)
                            nc.scalar.dma_start(
                                out.ap()[o * P:(o + 1) * P, n0 + H:n0 + nw],
                                ot[:, H:nw])
                        else:
                            nc.vector.scalar_tensor_tensor(
                                ot[:, 0:nw], pss[0][:],
                                bias_sb[:, o:o + 1], mask_sb[:, n0:n0 + nw],
                                op0=mybir.AluOpType.add,
                                op1=mybir.AluOpType.mult)
                            nc.scalar.dma_start(
                                out.ap()[o * P:(o + 1) * P, n0:n0 + nw],
                                ot[:])
                    else:
                        for j in range(nsub):
                            nj = n0 + j * NT
                            nc.vector.scalar_tensor_tensor(
                                ot_w[:, o, j * NT:(j + 1) * NT], pss[j][:],
                                bias_sb[:, o:o + 1], mask_sb[:, nj:nj + NT],
                                op0=mybir.AluOpType.add,
                                op1=mybir.AluOpType.mult)
                    # deferred bulk loads, unlocked by eviction progress
                    # (gate reads one element the chain's eviction wrote),
                    # spaced so each lands just before its deadline and
                    # never floods HBM alongside an earlier transfer.
                    if ns == 0 and o in (0, 2):
                        op = 2 + o // 2
                        gated_load(nc.scalar,
                                   w_sb[:, :, op * OP:(op + 1) * OP],
                                   wt_r[:, :, op * OP:(op + 1) * OP],
                                   ot_w[0:1, o, 0:1])
                    gate_w = None
                    if ns == 0 and o == 3:
                        gate_w = 1
                    elif ns == 0 and o == 7:
                        gate_w = 2
                    elif ns == 1 and o == 2:
                        gate_w = 3
                    elif ns == 1 and o == 6:
                        gate_w = 4
                    if gate_w is not None:
                        gw0 = win_off[gate_w]
                        gnw = WINDOWS[gate_w]
                        gated_load(nc.sync,
                                   x_sb[:, :, gw0:gw0 + gnw],
                                   x_r[:, :, gw0:gw0 + gnw],
                                   ot_w[0:1, o, 0:1])
                if ns != last_ns:
                    # one wide store for the whole window, gated by its
                    # last eviction (reads the full ot_w tile).
                    nc.scalar.dma_start(out_r[:, :, n0:n0 + nw], ot_w[:])
    nc.compile()
    return nc


_NC_CACHE = None


def _get_module():
    global _NC_CACHE
    if _NC_CACHE is None:
        _NC_CACHE = build_module()
    return _NC_CACHE


def _make_in_maps(cond, x_mask, W, b):
    bf16 = ml_dtypes.bfloat16
    wt = np.ascontiguousarray(
        W.astype(np.float32).sum(axis=0).T.astype(bf16))           # [C, O]
    bv = np.ascontiguousarray(
        b.astype(np.float32).sum(axis=0).reshape(OO, P).T,
        dtype=np.float32)                                          # [128, OO]
    in_maps = []
    for core in range(N_CORES):
        mrow = (x_mask[core, 0].astype(np.float32) / 16.0).astype(bf16)
        in_maps.append({
            "x": np.ascontiguousarray(cond[core].astype(bf16)),
            "wt": wt,
            "bv": bv,
            "mkr": np.ascontiguousarray(np.tile(mrow[None, :], (16, 1))),
        })
    return in_maps


def run(cond, x_mask, W, b, trace=False, trace_cores=None):
    """Run on hardware; returns (out [B,O,N] fp32, BassKernelResults)."""
    nc = _get_module()
    in_maps = _make_in_maps(cond, x_mask, W, b)
    res = run_bass_kernel_spmd(
        nc, in_maps, core_ids=list(range(N_CORES)),
        trace=trace, trace_cores=trace_cores,
    )
    out = np.stack(
        [res.results[i]["out"].astype(np.float32) for i in range(N_CORES)],
        axis=0)
    return out, res


def kernel(cond, x_mask, W, b):
    out, _ = run(cond, x_mask, W, b)
    return out


# revision 25
# speedup vs baseline: 1.1775x; 1.0007x over previous
# Trainium2 Bass kernel for nn_MultiCondLayer:
#   out[b,o,n] = (sum_k (cond[b] @ W[k].T)[o,n] + sum_k b[k,o]) * x_mask[b,0,n]
# Algebraic reduction: sum_k Linear_k(x) == Linear(x) with W' = sum_k W[k],
# b' = sum_k b[k]  (4x FLOP reduction vs. the naive einsum over k).
#
# Sharding: data-parallel over batch B=8 across the 8 NeuronCores (one batch
# element per core); the reduced [1024,1024] weight is replicated.
#
# Precision: all operands are cast to bf16 on the host (x, W', mask) and the
# output is stored bf16 and upcast on the host. PSUM accumulation stays fp32.
# The PE streams bf16 at the same 1 col/cycle as fp32r, so this does not
# change the ~110us matmul floor, but it halves HBM traffic 38->19 MB/core,
# enables FWL (fp32 LDWEIGHTS was ~330ns and leaked into the issue rate;
# bf16 is ~97ns, fully hidden), and halves the startup ramp and store tail.
# End-to-end rel err ~3e-3, well under the 2e-2 gate.
#
# Schedule notes (from perfetto/NTFF iterations):
# - The PE HAM clock gate needs ~3.4us of UNBROKEN matmul activity to go
#   1.2->2.4GHz, and any idle gap re-throttles it. So the kernel opens with
#   dummy matmuls on memset data (no DMA dependency), rolls into the mask
#   PE-broadcast, then into the main chains, with no PE gap anywhere.
# - All in-flight DMAs share the ~358 GB/s HBM limit round-robin regardless
#   of ring/queue, so the startup-critical set (mask row 8KB, x window 0
#   1MB, first w o-pair column 512KB) is issued alone; every other load is
#   gated behind a tiny 1-element memset placed on the vector queue (WAW
#   overlap -> the DMA's issue waits for it), timed so each window lands
#   just before its chains.
# - Outputs are evicted per chain by fused DVE (psum+bias)*mask into wide
#   per-window [128, 8o, nw] tiles and stored as ONE DMA per window
#   (automatically gated by the window's last eviction), so no store
#   traffic competes with the startup loads. The last window stores per
#   chain with a split final store to minimize the end-of-kernel tail.
# - Main stream: per n-window, 8 serial o-chains of 8 c-steps (LDW + 1-2
#   matmuls per step) accumulating into 1-2 psum banks; evictions stagger
#   and psum bank reuse has ~4 chains of slack. Measured issue rate:
#   216ns/matmul (the N=512 floor).

import numpy as np

import ml_dtypes

import concourse.bass as bass
import concourse.mybir as mybir
import concourse.tile as tile
from concourse import bacc
from concourse.bass_utils import run_bass_kernel_spmd

P = 128
B, C, N = 8, 1024, 4096
O = 1024
NT = 512                 # matmul free dim = one fp32 PSUM bank
CO, OO = C // P, O // P
# n-window plan: narrow first window so the startup-critical x DMA is only
# 1MB; narrow last window so the final evict+store tail is small.
WINDOWS = [512, 1024, 1024, 1024, 512]
F32 = mybir.dt.float32
BF16 = mybir.dt.bfloat16

N_CORES = 8
N_WARM = 10              # dummy matmuls to warm the HAM clock gate
OP = 2 * P               # w load granularity: o-pair columns (512KB)


def build_module():
    nc = bacc.Bacc("TRN2", target_bir_lowering=False, debug=False,
                   num_devices=N_CORES)
    x = nc.dram_tensor("x", [C, N], BF16, kind="ExternalInput")    # cond[b]
    wt = nc.dram_tensor("wt", [C, O], BF16, kind="ExternalInput")  # (sum_k W[k]).T
    # bias pre-transposed on host to [128, OO]: 128 contiguous rows.
    bv = nc.dram_tensor("bv", [P, OO], F32, kind="ExternalInput")
    # mask row, divided by 16 and replicated to 16 partitions on the host:
    # a 16-partition 128KB DMA completes fast and balanced (a 1-partition
    # row DMA's completion sem straggles behind bulk packets), and the PE
    # broadcast ones[16,:].T @ mkr sums the 16 copies back to the mask.
    mkr = nc.dram_tensor("mkr", [16, N], BF16, kind="ExternalInput")
    out = nc.dram_tensor("out", [O, N], BF16, kind="ExternalOutput")

    x_r = x.ap().rearrange("(c p) n -> p c n", p=P)      # [128, CO, N]
    wt_r = wt.ap().rearrange("(c p) o -> p c o", p=P)    # [128, CO, O]
    out_r = out.ap().rearrange("(o p) n -> p o n", p=P)  # [128, OO, N]

    win_off = []
    n0 = 0
    for nw in WINDOWS:
        win_off.append(n0)
        n0 += nw

    with tile.TileContext(nc) as tc:
        with (
            tc.tile_pool(name="consts", bufs=1) as consts,
            tc.tile_pool(name="outs", bufs=4) as outs,
            tc.tile_pool(name="ps", bufs=8, space="PSUM") as psp,
        ):
            # --- gpsimd: warmup memsets (they gate the PE ramp; gpsimd is
            # free earliest and issues no DMAs).
            scratch = consts.tile([P, NT], BF16)
            nc.gpsimd.memset(scratch[:], 0.0)
            ones_sb = consts.tile([16, P], BF16)
            nc.gpsimd.memset(ones_sb[:], 1.0)

            # --- startup-critical loads: mask row + bias at the scalar
            # ring head (tiny; done before the bulk builds up); x window 0
            # then the first w o-pair column on sync, which empirically
            # drains fastest -- within a ring each engine processes its
            # slots in FIFO order, so this is true priority ordering.
            mkr_sb = consts.tile([16, N], BF16)
            nc.scalar.dma_start(mkr_sb[:], mkr.ap())
            bias_sb = consts.tile([P, OO], F32)
            nc.scalar.dma_start(bias_sb[:], bv.ap())
            x_sb = consts.tile([P, CO, N], BF16)
            nc.sync.dma_start(x_sb[:, :, 0:WINDOWS[0]],
                              x_r[:, :, 0:WINDOWS[0]])
            w_sb = consts.tile([P, CO, O], BF16)
            nc.sync.dma_start(w_sb[:, :, 0:OP], wt_r[:, :, 0:OP])

            # Deferred-load helper: a 1-element vector copy that READS a
            # tile produced by `after` (a real data dependency -- the Tile
            # scheduler hoists dependency-free ops) and writes the DMA's
            # destination byte, so the DMA's issue waits (WAW) until
            # `after` has executed -- keeping bulk loads out of the
            # startup-critical HBM window.
            def gated_load(engine, dst, src, after_1elem):
                g = dst[0:1, 0, 0:1]
                nc.vector.tensor_copy(g, after_1elem)
                engine.dma_start(dst, src)

            # --- PE warmup: no-DMA dummy matmuls release the HAM throttle
            # (cold 1.2GHz -> warm 2.4GHz needs ~3.4us of UNBROKEN busy)
            # while the critical loads are in flight. ---
            for i in range(N_WARM):
                wps = psp.tile([P, NT], F32, name=f"warm_{i}", tag="ps")
                nc.tensor.matmul(wps[:], scratch[:, 0:P], scratch[:],
                                 start=True, stop=True)

            # --- Mask broadcast on-chip: ones[128,1] (x) mkrow[1,N] via PE
            # (keeps warming; avoids a 128x replicated mask DMA). Bulk w
            # columns and x window 1 unlock as the copies retire. ---
            mask_sb = consts.tile([P, N], BF16)
            for n in range(N // NT):
                mps = psp.tile([P, NT], F32, name=f"mps_{n}", tag="ps")
                nc.tensor.matmul(mps[:], ones_sb[:],
                                 mkr_sb[:, n * NT:(n + 1) * NT],
                                 start=True, stop=True)
                nc.vector.tensor_copy(mask_sb[:, n * NT:(n + 1) * NT], mps[:])
                if n == 0:
                    # second w o-pair column unlocks on the first mask copy
                    gated_load(nc.scalar, w_sb[:, :, OP:2 * OP],
                               wt_r[:, :, OP:2 * OP],
                               mask_sb[0:1, 0:1])

            # --- Main stream: per n-window, 8 serial o-chains of 8 c-steps.
            last_ns = len(WINDOWS) - 1
            for ns, nw in enumerate(WINDOWS):
                n0 = win_off[ns]
                nsub = nw // NT
                if ns != last_ns:
                    ot_w = outs.tile([P, OO, nw], BF16, name=f"otw_{ns}",
                                     tag=f"otw{nw}", bufs=2)
                for o in range(OO):
                    pss = [psp.tile([P, NT], F32, name=f"ps_{ns}_{o}_{j}",
                                    tag="ps") for j in range(nsub)]
                    for c in range(CO):
                        w_ap = w_sb[:, c, o * P:(o + 1) * P]
                        for j in range(nsub):
                            nj = n0 + j * NT
                            nc.tensor.matmul(pss[j][:], w_ap,
                                             x_sb[:, c, nj:nj + NT],
                                             start=(c == 0),
                                             stop=(c == CO - 1))
                    if ns == last_ns:
                        # last window: per-chain stores; split the final
                        # store so its completion receipt starts earlier.
                        ot = outs.tile([P, nw], BF16, name=f"ot_{ns}_{o}",
                                       tag="ot_last", bufs=4)
                        if o == OO - 1:
                            H = nw // 2
                            nc.vector.scalar_tensor_tensor(
                                ot[:, 0:H], pss[0][:, 0:H],
                                bias_sb[:, o:o + 1], mask_sb[:, n0:n0 + H],
                                op0=mybir.AluOpType.add,
                                op1=mybir.AluOpType.mult)
                            nc.scalar.dma_start(
                                out.ap()[o * P:(o + 1) * P, n0:n0 + H],
                                ot[:, 0:H])
                            nc.vector.scalar_tensor_tensor(
                                ot[:, H:nw], pss[0][:, H:nw],
                                bias_sb[:, o:o + 1],
                                mask_sb[:, n0 + H:n0 + nw],
                                op0=mybir.AluOpType.add,
                                op1=mybir.AluOpType.mult)
                            nc.scalar.dma_start(
                                out.ap()[o * P:(o + 1) * P, n0 + H:n0 + nw],
                                ot[:, H:nw])
                        else:
                            nc.vector.scalar_tensor_tensor(
                                ot[:, 0:nw], pss[0][:],
                                bias_sb[:, o:o + 1], mask_sb[:, n0:n0 + nw],
                                op0=mybir.AluOpType.add,
                                op1=mybir.AluOpType.mult)
                            nc.scalar.dma_start(
                                out.ap()[o * P:(o + 1) * P, n0:n0 + nw],
                                ot[:])
                    else:
                        for j in range(nsub):
                            nj = n0 + j * NT
                            nc.vector.scalar_tensor_tensor(
                                ot_w[:, o, j * NT:(j + 1) * NT], pss[j][:],
                                bias_sb[:, o:o + 1], mask_sb[:, nj:nj + NT],
                                op0=mybir.AluOpType.add,
                                op1=mybir.AluOpType.mult)
                    # deferred bulk loads, unlocked by eviction progress
                    # (gate reads one element the chain's eviction wrote),
                    # spaced so each lands just before its deadline and
                    # never floods HBM alongside an earlier transfer.
                    if ns == 0 and o in (0, 2):
                        op = 2 + o // 2
                        gated_load(nc.scalar,
                                   w_sb[:, :, op * OP:(op + 1) * OP],
                                   wt_r[:, :, op * OP:(op + 1) * OP],
                                   ot_w[0:1, o, 0:1])
                    gate_w = None
                    if ns == 0 and o == 3:
                        gate_w = 1
                    elif ns == 0 and o == 7:
                        gate_w = 2
                    elif ns == 1 and o == 2:
                        gate_w = 3
                    elif ns == 1 and o == 6:
                        gate_w = 4
                    if gate_w is not None:
                        gw0 = win_off[gate_w]
                        gnw = WINDOWS[gate_w]
                        gated_load(nc.sync,
                                   x_sb[:, :, gw0:gw0 + gnw],
                                   x_r[:, :, gw0:gw0 + gnw],
                                   ot_w[0:1, o, 0:1])
                if ns != last_ns:
                    # one wide store for the whole window, gated by its
                    # last eviction (reads the full ot_w tile).
                    nc.scalar.dma_start(out_r[:, :, n0:n0 + nw], ot_w[:])
    nc.compile()
    return nc


_NC_CACHE = None


def _get_module():
    global _NC_CACHE
    if _NC_CACHE is None:
        _NC_CACHE = build_module()
    return _NC_CACHE


def _make_in_maps(cond, x_mask, W, b):
    bf16 = ml_dtypes.bfloat16
    wt = np.ascontiguousarray(
        W.astype(np.float32).sum(axis=0).T.astype(bf16))           # [C, O]
    bv = np.ascontiguousarray(
        b.astype(np.float32).sum(axis=0).reshape(OO, P).T,
        dtype=np.float32)                                          # [128, OO]
    in_maps = []
    for core in range(N_CORES):
        mrow = (x_mask[core, 0].astype(np.float32) / 16.0).astype(bf16)
        in_maps.append({
            "x": np.ascontiguousarray(cond[core].astype(bf16)),
            "wt": wt,
            "bv": bv,
            "mkr": np.ascontiguousarray(np.tile(mrow[None, :], (16, 1))),
        })
    return in_maps


def run(cond, x_mask, W, b, trace=False, trace_cores=None):
    """Run on hardware; returns (out [B,O,N] fp32, BassKernelResults)."""
    nc = _get_module()
    in_maps = _make_in_maps(cond, x_mask, W, b)
    res = run_bass_kernel_spmd(
        nc, in_maps, core_ids=list(range(N_CORES)),
        trace=trace, trace_cores=trace_cores,
    )
    out = np.stack(
        [res.results[i]["out"].astype(np.float32) for i in range(N_CORES)],
        axis=0)
    return out, res


def kernel(cond, x_mask, W, b):
    out, _ = run(cond, x_mask, W, b)
    return out


# revision 27
# speedup vs baseline: 1.2329x; 1.0470x over previous
# Trainium2 Bass kernel for nn_MultiCondLayer:
#   out[b,o,n] = (sum_k (cond[b] @ W[k].T)[o,n] + sum_k b[k,o]) * x_mask[b,0,n]
# Algebraic reduction: sum_k Linear_k(x) == Linear(x) with W' = sum_k W[k],
# b' = sum_k b[k]  (4x FLOP reduction vs. the naive einsum over k).
#
# Sharding: data-parallel over batch B=8 across the 8 NeuronCores (one batch
# element per core); the reduced [1024,1024] weight is replicated.
#
# Precision: all operands are cast to bf16 on the host (x, W', mask) and the
# output is stored bf16 and upcast on the host. PSUM accumulation stays fp32.
# The PE streams bf16 at the same 1 col/cycle as fp32r, so this does not
# change the ~110us matmul floor, but it halves HBM traffic 38->19 MB/core,
# enables FWL (fp32 LDWEIGHTS was ~330ns and leaked into the issue rate;
# bf16 is ~97ns, fully hidden), and halves the startup ramp and store tail.
# End-to-end rel err ~3e-3, well under the 2e-2 gate.
#
# Schedule notes (from perfetto/NTFF iterations):
# - The PE HAM clock gate needs ~3.4us of UNBROKEN matmul activity to go
#   1.2->2.4GHz, and any idle gap re-throttles it. So the kernel opens with
#   dummy matmuls on memset data (no DMA dependency), rolls into the mask
#   PE-broadcast, then into the main chains, with no PE gap anywhere.
# - All in-flight DMAs share the ~358 GB/s HBM limit round-robin regardless
#   of ring/queue, so the startup-critical set (mask row 8KB, x window 0
#   1MB, first w o-pair column 512KB) is issued alone; every other load is
#   gated behind a tiny 1-element memset placed on the vector queue (WAW
#   overlap -> the DMA's issue waits for it), timed so each window lands
#   just before its chains.
# - Outputs are evicted per chain by fused DVE (psum+bias)*mask into wide
#   per-window [128, 8o, nw] tiles and stored as ONE DMA per window
#   (automatically gated by the window's last eviction), so no store
#   traffic competes with the startup loads. The last window stores per
#   chain with a split final store to minimize the end-of-kernel tail.
# - Main stream: per n-window, 8 serial o-chains of 8 c-steps (LDW + 1-2
#   matmuls per step) accumulating into 1-2 psum banks; evictions stagger
#   and psum bank reuse has ~4 chains of slack. Measured issue rate:
#   216ns/matmul (the N=512 floor).

import numpy as np

import ml_dtypes

import concourse.bass as bass
import concourse.mybir as mybir
import concourse.tile as tile
from concourse import bacc
from concourse.bass_utils import run_bass_kernel_spmd

P = 128
B, C, N = 8, 1024, 4096
O = 1024
NT = 512                 # matmul free dim = one fp32 PSUM bank
CO, OO = C // P, O // P
# n-window plan: narrow first window so the startup-critical x DMA is only
# 1MB; narrow last window so the final evict+store tail is small.
WINDOWS = [512, 1024, 1024, 1024, 512]
F32 = mybir.dt.float32
BF16 = mybir.dt.bfloat16

N_CORES = 8
N_WARM = 10              # dummy matmuls to warm the HAM clock gate
OP = 2 * P               # w load granularity: o-pair columns (512KB)


def build_module():
    nc = bacc.Bacc("TRN2", target_bir_lowering=False, debug=False,
                   num_devices=N_CORES)
    x = nc.dram_tensor("x", [C, N], BF16, kind="ExternalInput")    # cond[b]
    wt = nc.dram_tensor("wt", [C, O], BF16, kind="ExternalInput")  # (sum_k W[k]).T
    # bias pre-transposed on host to [128, OO]: 128 contiguous rows.
    bv = nc.dram_tensor("bv", [P, OO], F32, kind="ExternalInput")
    # mask row, divided by 16 and replicated to 16 partitions on the host:
    # a 16-partition 128KB DMA completes fast and balanced (a 1-partition
    # row DMA's completion sem straggles behind bulk packets), and the PE
    # broadcast ones[16,:].T @ mkr sums the 16 copies back to the mask.
    mkr = nc.dram_tensor("mkr", [16, N], BF16, kind="ExternalInput")
    out = nc.dram_tensor("out", [O, N], BF16, kind="ExternalOutput")

    x_r = x.ap().rearrange("(c p) n -> p c n", p=P)      # [128, CO, N]
    wt_r = wt.ap().rearrange("(c p) o -> p c o", p=P)    # [128, CO, O]
    out_r = out.ap().rearrange("(o p) n -> p o n", p=P)  # [128, OO, N]

    win_off = []
    n0 = 0
    for nw in WINDOWS:
        win_off.append(n0)
        n0 += nw

    with tile.TileContext(nc) as tc:
        with (
            tc.tile_pool(name="consts", bufs=1) as consts,
            tc.tile_pool(name="outs", bufs=4) as outs,
            tc.tile_pool(name="ps", bufs=8, space="PSUM") as psp,
        ):
            # --- gpsimd: warmup memsets (they gate the PE ramp; gpsimd is
            # free earliest and issues no DMAs).
            scratch = consts.tile([P, NT], BF16)
            nc.gpsimd.memset(scratch[:], 0.0)
            ones_sb = consts.tile([16, P], BF16)
            nc.gpsimd.memset(ones_sb[:], 1.0)

            # --- startup-critical loads: mask row + bias at the scalar
            # ring head (tiny; done before the bulk builds up); x window 0
            # then the first w o-pair column on sync, which empirically
            # drains fastest -- within a ring each engine processes its
            # slots in FIFO order, so this is true priority ordering.
            mkr_sb = consts.tile([16, N], BF16)
            nc.scalar.dma_start(mkr_sb[:], mkr.ap())
            bias_sb = consts.tile([P, OO], F32)
            nc.scalar.dma_start(bias_sb[:], bv.ap())
            x_sb = consts.tile([P, CO, N], BF16)
            nc.sync.dma_start(x_sb[:, :, 0:WINDOWS[0]],
                              x_r[:, :, 0:WINDOWS[0]])
            w_sb = consts.tile([P, CO, O], BF16)
            nc.sync.dma_start(w_sb[:, :, 0:OP], wt_r[:, :, 0:OP])

            # Deferred-load helper: a 1-element vector copy that READS a
            # tile produced by `after` (a real data dependency -- the Tile
            # scheduler hoists dependency-free ops) and writes the DMA's
            # destination byte, so the DMA's issue waits (WAW) until
            # `after` has executed -- keeping bulk loads out of the
            # startup-critical HBM window.
            def gated_load(engine, dst, src, after_1elem):
                g = dst[0:1, 0, 0:1]
                nc.vector.tensor_copy(g, after_1elem)
                engine.dma_start(dst, src)

            # --- PE warmup: no-DMA dummy matmuls release the HAM throttle
            # (cold 1.2GHz -> warm 2.4GHz needs ~3.4us of UNBROKEN busy)
            # while the critical loads are in flight. ---
            for i in range(N_WARM):
                wps = psp.tile([P, NT], F32, name=f"warm_{i}", tag="ps")
                nc.tensor.matmul(wps[:], scratch[:, 0:P], scratch[:],
                                 start=True, stop=True)

            # --- Mask broadcast on-chip: ones[128,1] (x) mkrow[1,N] via PE
            # (keeps warming; avoids a 128x replicated mask DMA). Bulk w
            # columns and x window 1 unlock as the copies retire. ---
            mask_sb = consts.tile([P, N], BF16)
            for n in range(N // NT):
                mps = psp.tile([P, NT], F32, name=f"mps_{n}", tag="ps")
                nc.tensor.matmul(mps[:], ones_sb[:],
                                 mkr_sb[:, n * NT:(n + 1) * NT],
                                 start=True, stop=True)
                nc.vector.tensor_copy(mask_sb[:, n * NT:(n + 1) * NT], mps[:])
                if n == 0:
                    # second w o-pair column unlocks on the first mask copy
                    gated_load(nc.scalar, w_sb[:, :, OP:2 * OP],
                               wt_r[:, :, OP:2 * OP],
                               mask_sb[0:1, 0:1])
                elif n == 4:
                    # upper w half (o4-o7): needed from chain o4 (~7us in)
                    gated_load(nc.scalar, w_sb[:, :, 4 * P:O],
                               wt_r[:, :, 4 * P:O],
                               mask_sb[0:1, 4 * NT:4 * NT + 1])

            # --- Main stream: per n-window, 8 serial o-chains of 8 c-steps.
            last_ns = len(WINDOWS) - 1
            for ns, nw in enumerate(WINDOWS):
                n0 = win_off[ns]
                nsub = nw // NT
                if ns != last_ns:
                    ot_w = outs.tile([P, OO, nw], BF16, name=f"otw_{ns}",
                                     tag=f"otw{nw}", bufs=2)
                for o in range(OO):
                    pss = [psp.tile([P, NT], F32, name=f"ps_{ns}_{o}_{j}",
                                    tag="ps") for j in range(nsub)]
                    for c in range(CO):
                        w_ap = w_sb[:, c, o * P:(o + 1) * P]
                        for j in range(nsub):
                            nj = n0 + j * NT
                            nc.tensor.matmul(pss[j][:], w_ap,
                                             x_sb[:, c, nj:nj + NT],
                                             start=(c == 0),
                                             stop=(c == CO - 1))
                    if ns == last_ns:
                        # last window: per-chain stores; split the final
                        # store so its completion receipt starts earlier.
                        ot = outs.tile([P, nw], BF16, name=f"ot_{ns}_{o}",
                                       tag="ot_last", bufs=4)
                        if o == OO - 1:
                            H = nw // 2
                            nc.vector.scalar_tensor_tensor(
                                ot[:, 0:H], pss[0][:, 0:H],
                                bias_sb[:, o:o + 1], mask_sb[:, n0:n0 + H],
                                op0=mybir.AluOpType.add,
                                op1=mybir.AluOpType.mult)
                            nc.scalar.dma_start(
                                out.ap()[o * P:(o + 1) * P, n0:n0 + H],
                                ot[:, 0:H])
                            nc.vector.scalar_tensor_tensor(
                                ot[:, H:nw], pss[0][:, H:nw],
                                bias_sb[:, o:o + 1],
                                mask_sb[:, n0 + H:n0 + nw],
                                op0=mybir.AluOpType.add,
                                op1=mybir.AluOpType.mult)
                            nc.scalar.dma_start(
                                out.ap()[o * P:(o + 1) * P, n0 + H:n0 + nw],
                                ot[:, H:nw])
                        else:
                            nc.vector.scalar_tensor_tensor(
                                ot[:, 0:nw], pss[0][:],
                                bias_sb[:, o:o + 1], mask_sb[:, n0:n0 + nw],
                                op0=mybir.AluOpType.add,
                                op1=mybir.AluOpType.mult)
                            nc.scalar.dma_start(
                                out.ap()[o * P:(o + 1) * P, n0:n0 + nw],
                                ot[:])
                    else:
                        for j in range(nsub):
                            nj = n0 + j * NT
                            nc.vector.scalar_tensor_tensor(
                                ot_w[:, o, j * NT:(j + 1) * NT], pss[j][:],
                                bias_sb[:, o:o + 1], mask_sb[:, nj:nj + NT],
                                op0=mybir.AluOpType.add,
                                op1=mybir.AluOpType.mult)
                    # deferred bulk x loads, unlocked by eviction progress
                    # (gate reads one element the chain's eviction wrote),
                    # each opening ~5us before its deadline: gate-to-
                    # usable is roughly issue + shared transfer + sem.
                    gate_w = None
                    if ns == 0 and o == 1:
                        gate_w = 1
                    elif ns == 0 and o == 5:
                        gate_w = 2
                    elif ns == 1 and o == 1:
                        gate_w = 3
                    elif ns == 1 and o == 5:
                        gate_w = 4
                    if gate_w is not None:
                        gw0 = win_off[gate_w]
                        gnw = WINDOWS[gate_w]
                        gated_load(nc.sync,
                                   x_sb[:, :, gw0:gw0 + gnw],
                                   x_r[:, :, gw0:gw0 + gnw],
                                   ot_w[0:1, o, 0:1])
                if ns != last_ns:
                    # one wide store for the whole window, gated by its
                    # last eviction (reads the full ot_w tile).
                    nc.scalar.dma_start(out_r[:, :, n0:n0 + nw], ot_w[:])
    nc.compile()
    return nc


_NC_CACHE = None


def _get_module():
    global _NC_CACHE
    if _NC_CACHE is None:
        _NC_CACHE = build_module()
    return _NC_CACHE


def _make_in_maps(cond, x_mask, W, b):
    bf16 = ml_dtypes.bfloat16
    wt = np.ascontiguousarray(
        W.astype(np.float32).sum(axis=0).T.astype(bf16))           # [C, O]
    bv = np.ascontiguousarray(
        b.astype(np.float32).sum(axis=0).reshape(OO, P).T,
        dtype=np.float32)                                          # [128, OO]
    in_maps = []
    for core in range(N_CORES):
        mrow = (x_mask[core, 0].astype(np.float32) / 16.0).astype(bf16)
        in_maps.append({
            "x": np.ascontiguousarray(cond[core].astype(bf16)),
            "wt": wt,
            "bv": bv,
            "mkr": np.ascontiguousarray(np.tile(mrow[None, :], (16, 1))),
        })
    return in_maps


def run(cond, x_mask, W, b, trace=False, trace_cores=None):
    """Run on hardware; returns (out [B,O,N] fp32, BassKernelResults)."""
    nc = _get_module()
    in_maps = _make_in_maps(cond, x_mask, W, b)
    res = run_bass_kernel_spmd(
        nc, in_maps, core_ids=list(range(N_CORES)),
        trace=trace, trace_cores=trace_cores,
    )
    out = np.stack(
        [res.results[i]["out"].astype(np.float32) for i in range(N_CORES)],
        axis=0)
    return out, res


def kernel(cond, x_mask, W, b):
    out, _ = run(cond, x_mask, W, b)
    return out


# revision 30
# speedup vs baseline: 1.2573x; 1.0198x over previous
# Trainium2 Bass kernel for nn_MultiCondLayer:
#   out[b,o,n] = (sum_k (cond[b] @ W[k].T)[o,n] + sum_k b[k,o]) * x_mask[b,0,n]
# Algebraic reduction: sum_k Linear_k(x) == Linear(x) with W' = sum_k W[k],
# b' = sum_k b[k]  (4x FLOP reduction vs. the naive einsum over k).
#
# Sharding: data-parallel over batch B=8 across the 8 NeuronCores (one batch
# element per core); the reduced [1024,1024] weight is replicated.
#
# Precision: all operands are cast to bf16 on the host (x, W', mask) and the
# output is stored bf16 and upcast on the host. PSUM accumulation stays fp32.
# The PE streams bf16 at the same 1 col/cycle as fp32r, so this does not
# change the ~110us matmul floor, but it halves HBM traffic 38->19 MB/core,
# enables FWL (fp32 LDWEIGHTS was ~330ns and leaked into the issue rate;
# bf16 is ~97ns, fully hidden), and halves the startup ramp and store tail.
# End-to-end rel err ~3e-3, well under the 2e-2 gate.
#
# Schedule notes (from perfetto/NTFF iterations):
# - The PE HAM clock gate needs ~3.4us of UNBROKEN matmul activity to go
#   1.2->2.4GHz, and any idle gap re-throttles it. So the kernel opens with
#   dummy matmuls on memset data (no DMA dependency), rolls into the mask
#   PE-broadcast, then into the main chains, with no PE gap anywhere.
# - All in-flight DMAs share the ~358 GB/s HBM limit round-robin regardless
#   of ring/queue, so the startup-critical set (mask row 8KB, x window 0
#   1MB, first w o-pair column 512KB) is issued alone; every other load is
#   gated behind a tiny 1-element memset placed on the vector queue (WAW
#   overlap -> the DMA's issue waits for it), timed so each window lands
#   just before its chains.
# - Outputs are evicted per chain by fused DVE (psum+bias)*mask into wide
#   per-window [128, 8o, nw] tiles and stored as ONE DMA per window
#   (automatically gated by the window's last eviction), so no store
#   traffic competes with the startup loads. The last window stores per
#   chain with a split final store to minimize the end-of-kernel tail.
# - Main stream: per n-window, 8 serial o-chains of 8 c-steps (LDW + 1-2
#   matmuls per step) accumulating into 1-2 psum banks; evictions stagger
#   and psum bank reuse has ~4 chains of slack. Measured issue rate:
#   216ns/matmul (the N=512 floor).

import numpy as np

import ml_dtypes

import concourse.bass as bass
import concourse.mybir as mybir
import concourse.tile as tile
from concourse import bacc
from concourse.bass_utils import run_bass_kernel_spmd

P = 128
B, C, N = 8, 1024, 4096
O = 1024
NT = 512                 # matmul free dim = one fp32 PSUM bank
CO, OO = C // P, O // P
# n-window plan: narrow first window so the startup-critical x DMA is only
# 1MB; narrow last window so the final evict+store tail is small.
WINDOWS = [512, 1024, 1024, 1024, 512]
F32 = mybir.dt.float32
BF16 = mybir.dt.bfloat16

N_CORES = 8
N_WARM = 12              # dummy matmuls to warm the HAM clock gate
OP = 2 * P               # w load granularity: o-pair columns (512KB)


def build_module():
    nc = bacc.Bacc("TRN2", target_bir_lowering=False, debug=False,
                   num_devices=N_CORES)
    x = nc.dram_tensor("x", [C, N], BF16, kind="ExternalInput")    # cond[b]
    wt = nc.dram_tensor("wt", [C, O], BF16, kind="ExternalInput")  # (sum_k W[k]).T
    # bias pre-transposed on host to [128, OO]: 128 contiguous rows.
    bv = nc.dram_tensor("bv", [P, OO], F32, kind="ExternalInput")
    # mask row, divided by 16 and replicated to 16 partitions on the host:
    # a 16-partition 128KB DMA completes fast and balanced (a 1-partition
    # row DMA's completion sem straggles behind bulk packets), and the PE
    # broadcast ones[16,:].T @ mkr sums the 16 copies back to the mask.
    mkr = nc.dram_tensor("mkr", [16, N], BF16, kind="ExternalInput")
    out = nc.dram_tensor("out", [O, N], BF16, kind="ExternalOutput")

    x_r = x.ap().rearrange("(c p) n -> p c n", p=P)      # [128, CO, N]
    wt_r = wt.ap().rearrange("(c p) o -> p c o", p=P)    # [128, CO, O]
    out_r = out.ap().rearrange("(o p) n -> p o n", p=P)  # [128, OO, N]

    win_off = []
    n0 = 0
    for nw in WINDOWS:
        win_off.append(n0)
        n0 += nw

    with tile.TileContext(nc) as tc:
        with (
            tc.tile_pool(name="consts", bufs=1) as consts,
            tc.tile_pool(name="outs", bufs=4) as outs,
            tc.tile_pool(name="ps", bufs=8, space="PSUM") as psp,
        ):
            # --- gpsimd: warmup memsets (they gate the PE ramp; gpsimd is
            # free earliest and issues no DMAs).
            scratch = consts.tile([P, NT], BF16)
            nc.gpsimd.memset(scratch[:], 0.0)
            ones_sb = consts.tile([16, P], BF16)
            nc.gpsimd.memset(ones_sb[:], 1.0)

            # --- startup-critical loads, in FIFO priority order on the
            # sync ring (which empirically drains fastest): mask row
            # first (it gates the mask broadcast that keeps the PE ramp
            # unbroken), then x window 0, then the first w o-pair column.
            # Bias at the scalar ring head.
            mkr_sb = consts.tile([16, N], BF16)
            nc.sync.dma_start(mkr_sb[:], mkr.ap())
            bias_sb = consts.tile([P, OO], F32)
            nc.scalar.dma_start(bias_sb[:], bv.ap())
            x_sb = consts.tile([P, CO, N], BF16)
            nc.sync.dma_start(x_sb[:, :, 0:WINDOWS[0]],
                              x_r[:, :, 0:WINDOWS[0]])
            w_sb = consts.tile([P, CO, O], BF16)
            nc.sync.dma_start(w_sb[:, :, 0:OP], wt_r[:, :, 0:OP])

            # Deferred-load helper: a 1-element vector copy that READS a
            # tile produced by `after` (a real data dependency -- the Tile
            # scheduler hoists dependency-free ops) and writes the DMA's
            # destination byte, so the DMA's issue waits (WAW) until
            # `after` has executed -- keeping bulk loads out of the
            # startup-critical HBM window.
            def gated_load(engine, dst, src, after_1elem):
                g = dst[0:1, 0, 0:1]
                nc.vector.tensor_copy(g, after_1elem)
                engine.dma_start(dst, src)

            # --- PE warmup: no-DMA dummy matmuls release the HAM throttle
            # (cold 1.2GHz -> warm 2.4GHz needs ~3.4us of UNBROKEN busy)
            # while the critical loads are in flight. ---
            for i in range(N_WARM):
                wps = psp.tile([P, NT], F32, name=f"warm_{i}", tag="ps")
                nc.tensor.matmul(wps[:], scratch[:, 0:P], scratch[:],
                                 start=True, stop=True)

            # --- Mask broadcast on-chip: ones[128,1] (x) mkrow[1,N] via PE
            # (keeps warming; avoids a 128x replicated mask DMA). Bulk w
            # columns and x window 1 unlock as the copies retire. ---
            mask_sb = consts.tile([P, N], BF16)
            for n in range(N // NT):
                mps = psp.tile([P, NT], F32, name=f"mps_{n}", tag="ps")
                nc.tensor.matmul(mps[:], ones_sb[:],
                                 mkr_sb[:, n * NT:(n + 1) * NT],
                                 start=True, stop=True)
                nc.vector.tensor_copy(mask_sb[:, n * NT:(n + 1) * NT], mps[:])
                if n == 0:
                    # second w o-pair column unlocks on the first mask copy
                    gated_load(nc.scalar, w_sb[:, :, OP:2 * OP],
                               wt_r[:, :, OP:2 * OP],
                               mask_sb[0:1, 0:1])
                elif n == 4:
                    # upper w half (o4-o7): needed from chain o4 (~7us in)
                    gated_load(nc.scalar, w_sb[:, :, 4 * P:O],
                               wt_r[:, :, 4 * P:O],
                               mask_sb[0:1, 4 * NT:4 * NT + 1])

            # --- post-mask warm filler: the critical x/w loads land at
            # ~14.5us; keep the PE busy (HAM re-throttles on ANY gap)
            # until the first chain can run without stalling. ---
            for i in range(4):
                fps = psp.tile([P, NT], F32, name=f"fill_{i}", tag="ps")
                nc.tensor.matmul(fps[:], scratch[:, 0:P], scratch[:],
                                 start=True, stop=True)

            # --- Main stream: per n-window, 8 serial o-chains of 8 c-steps.
            last_ns = len(WINDOWS) - 1
            for ns, nw in enumerate(WINDOWS):
                n0 = win_off[ns]
                nsub = nw // NT
                if ns != last_ns:
                    ot_w = outs.tile([P, OO, nw], BF16, name=f"otw_{ns}",
                                     tag=f"otw{nw}", bufs=2)
                for o in range(OO):
                    pss = [psp.tile([P, NT], F32, name=f"ps_{ns}_{o}_{j}",
                                    tag="ps") for j in range(nsub)]
                    for c in range(CO):
                        w_ap = w_sb[:, c, o * P:(o + 1) * P]
                        for j in range(nsub):
                            nj = n0 + j * NT
                            nc.tensor.matmul(pss[j][:], w_ap,
                                             x_sb[:, c, nj:nj + NT],
                                             start=(c == 0),
                                             stop=(c == CO - 1))
                    if ns == last_ns:
                        # last window: per-chain stores; split the final
                        # store so its completion receipt starts earlier.
                        ot = outs.tile([P, nw], BF16, name=f"ot_{ns}_{o}",
                                       tag="ot_last", bufs=4)
                        if o == OO - 1:
                            H = nw // 2
                            nc.vector.scalar_tensor_tensor(
                                ot[:, 0:H], pss[0][:, 0:H],
                                bias_sb[:, o:o + 1], mask_sb[:, n0:n0 + H],
                                op0=mybir.AluOpType.add,
                                op1=mybir.AluOpType.mult)
                            nc.scalar.dma_start(
                                out.ap()[o * P:(o + 1) * P, n0:n0 + H],
                                ot[:, 0:H])
                            nc.vector.scalar_tensor_tensor(
                                ot[:, H:nw], pss[0][:, H:nw],
                                bias_sb[:, o:o + 1],
                                mask_sb[:, n0 + H:n0 + nw],
                                op0=mybir.AluOpType.add,
                                op1=mybir.AluOpType.mult)
                            nc.scalar.dma_start(
                                out.ap()[o * P:(o + 1) * P, n0 + H:n0 + nw],
                                ot[:, H:nw])
                        else:
                            nc.vector.scalar_tensor_tensor(
                                ot[:, 0:nw], pss[0][:],
                                bias_sb[:, o:o + 1], mask_sb[:, n0:n0 + nw],
                                op0=mybir.AluOpType.add,
                                op1=mybir.AluOpType.mult)
                            nc.scalar.dma_start(
                                out.ap()[o * P:(o + 1) * P, n0:n0 + nw],
                                ot[:])
                    else:
                        for j in range(nsub):
                            nj = n0 + j * NT
                            nc.vector.scalar_tensor_tensor(
                                ot_w[:, o, j * NT:(j + 1) * NT], pss[j][:],
                                bias_sb[:, o:o + 1], mask_sb[:, nj:nj + NT],
                                op0=mybir.AluOpType.add,
                                op1=mybir.AluOpType.mult)
                    # deferred bulk x loads, unlocked by eviction progress
                    # (gate reads one element the chain's eviction wrote),
                    # each opening ~5us before its deadline: gate-to-
                    # usable is roughly issue + shared transfer + sem.
                    gate_w = None
                    if ns == 0 and o == 1:
                        gate_w = 1
                    elif ns == 0 and o == 5:
                        gate_w = 2
                    elif ns == 1 and o == 1:
                        gate_w = 3
                    elif ns == 1 and o == 5:
                        gate_w = 4
                    if gate_w is not None:
                        gw0 = win_off[gate_w]
                        gnw = WINDOWS[gate_w]
                        gated_load(nc.sync,
                                   x_sb[:, :, gw0:gw0 + gnw],
                                   x_r[:, :, gw0:gw0 + gnw],
                                   ot_w[0:1, o, 0:1])
                if ns != last_ns:
                    # one wide store for the whole window, gated by its
                    # last eviction (reads the full ot_w tile).
                    nc.scalar.dma_start(out_r[:, :, n0:n0 + nw], ot_w[:])
    nc.compile()
    return nc


_NC_CACHE = None


def _get_module():
    global _NC_CACHE
    if _NC_CACHE is None:
        _NC_CACHE = build_module()
    return _NC_CACHE


def _make_in_maps(cond, x_mask, W, b):
    bf16 = ml_dtypes.bfloat16
    wt = np.ascontiguousarray(
        W.astype(np.float32).sum(axis=0).T.astype(bf16))           # [C, O]
    bv = np.ascontiguousarray(
        b.astype(np.float32).sum(axis=0).reshape(OO, P).T,
        dtype=np.float32)                                          # [128, OO]
    in_maps = []
    for core in range(N_CORES):
        mrow = (x_mask[core, 0].astype(np.float32) / 16.0).astype(bf16)
        in_maps.append({
            "x": np.ascontiguousarray(cond[core].astype(bf16)),
            "wt": wt,
            "bv": bv,
            "mkr": np.ascontiguousarray(np.tile(mrow[None, :], (16, 1))),
        })
    return in_maps


def run(cond, x_mask, W, b, trace=False, trace_cores=None):
    """Run on hardware; returns (out [B,O,N] fp32, BassKernelResults)."""
    nc = _get_module()
    in_maps = _make_in_maps(cond, x_mask, W, b)
    res = run_bass_kernel_spmd(
        nc, in_maps, core_ids=list(range(N_CORES)),
        trace=trace, trace_cores=trace_cores,
    )
    out = np.stack(
        [res.results[i]["out"].astype(np.float32) for i in range(N_CORES)],
        axis=0)
    return out, res


def kernel(cond, x_mask, W, b):
    out, _ = run(cond, x_mask, W, b)
    return out
